# revision 5
# baseline (speedup 1.0000x reference)
"""Trainium2 Bass kernel for nn_DDPM_39831526703349.

Strategy: pure data parallel over batch N=8 -> 1 image per NeuronCore.
Each conv is lowered to per-tap matmuls over channel tiles with accumulation
in PSUM; BN is folded into weights host-side; bias/BN-shift/ReLU applied by
ScalarE on PSUM->SBUF eviction. Activations live in SBUF in a zero-padded
layout so 3x3 (and dilated) taps are pure AP shifts. The per-pixel dynamic
conv runs on VectorE against the padded xd buffer. Matmul operands are bf16
(fp32 accumulation in PSUM); biases and the final output are fp32.
"""

import os
import numpy as np
import ml_dtypes
from contextlib import ExitStack

import concourse.bass as bass
import concourse.mybir as mybir
import concourse.tile as tile
from concourse.alu_op_type import AluOpType
from concourse.bass_utils import run_bass_kernel_spmd

bf16 = ml_dtypes.bfloat16
FP32 = mybir.dt.float32
BF16 = mybir.dt.bfloat16
ACT_F = mybir.ActivationFunctionType

H = W = 64
HP = WP = 66          # pad-1 layout
XP = 74               # pad-5 layout for xd
EPS = 1e-5
N_CORES = 8


# ---------------------------------------------------------------------------
# sync-overflow post-pass: this walrus accepts at most one sync wait and one
# sync update per instruction; split extras into same-engine EventSemaphores.
# ---------------------------------------------------------------------------
def _fix_sync_overflow(nc, max_waits=1, max_updates=1):
    uid = 0
    n_split = 0
    for f in nc.m.functions:
        for bb in f.blocks:
            insts = list(bb.instructions)
            out = []
            changed = False
            for ins in insts:
                si = ins.sync_info
                waits = list(si.on_wait) if (si and si.on_wait) else []
                updates = list(si.on_update) if (si and si.on_update) else []
                if len(waits) <= max_waits and len(updates) <= max_updates:
                    out.append(ins)
                    continue
                changed = True
                n_split += 1
                keep_w = waits[-max_waits:] if max_waits else []
                excess_w = waits[:-max_waits] if max_waits else waits
                keep_u = updates[:max_updates]
                excess_u = updates[max_updates:]
                for wt in excess_w:
                    uid += 1
                    ev = mybir.InstEventSemaphore(name=f"syncfix_w_{ins.name}_{uid}")
                    ev.engine = ins.engine
                    ev.sync_info = mybir.SyncInfo(on_wait=[wt], on_update=[])
                    out.append(ev)
                ins.sync_info = mybir.SyncInfo(on_wait=keep_w, on_update=keep_u)
                out.append(ins)
                for ud in excess_u:
                    uid += 1
                    ev = mybir.InstEventSemaphore(name=f"syncfix_u_{ins.name}_{uid}")
                    ev.engine = ins.engine
                    ev.sync_info = mybir.SyncInfo(on_wait=[], on_update=[ud])
                    out.append(ev)
            if changed:
                bb.instructions = out
    return n_split


# ---------------------------------------------------------------------------
# host-side weight preparation
# ---------------------------------------------------------------------------
def _np(a):
    return np.asarray(a, dtype=np.float32)


def _fold_basic(p):
    """BasicConv2d: fold BN scale into conv weights; return (w_folded, bias)."""
    w = _np(p['w'])
    bnp = p['bn']
    inv = (1.0 / np.sqrt(_np(bnp['var']) + EPS)) * _np(bnp['gamma'])
    beta = _np(bnp['beta']) - _np(bnp['mean']) * inv
    return w * inv[:, None, None, None], beta


def _lhsT_3x3(w):
    """w [O, I, 3, 3] -> [co_t, ci_t, 128, 9*128] bf16 (lhsT per tap)."""
    O, I = w.shape[0], w.shape[1]
    full = w.transpose(1, 2, 3, 0).reshape(I, 9, O)          # [ci, t, co]
    arr = full.reshape(I // 128, 128, 9, O // 128, 128)
    arr = arr.transpose(3, 0, 1, 2, 4)                        # [co_t, ci_t, 128, 9, 128]
    return np.ascontiguousarray(arr.reshape(O // 128, I // 128, 128, 9 * 128)).astype(bf16)


def _lhsT_1x1(w):
    """w [O, I, 1, 1] -> [co_t, ci_t, 128, 128] bf16."""
    O, I = w.shape[0], w.shape[1]
    full = w[:, :, 0, 0].T                                    # [ci, co]
    arr = full.reshape(I // 128, 128, O // 128, 128).transpose(2, 0, 1, 3)
    return np.ascontiguousarray(arr).astype(bf16)


def _prep_branch(p):
    """Prepare one DepthDC branch's weights/biases."""
    d = {}
    g0w = _np(p['g0_w'])
    d['g0'] = _lhsT_3x3(g0w)                                  # [4,4,128,1152]
    d['g0_b'] = _np(p['g0_b'])
    dn = p['dense']
    d['dsdown'] = _lhsT_1x1(_np(dn['down_w']))                # [1,4,128,128]
    d['dsdown_b'] = _np(dn['down_b'])
    d['blk'] = []
    d['blk_b'] = []
    for bp in dn['blocks']:
        w, b = _fold_basic(bp)
        d['blk'].append(_lhsT_3x3(w))                         # [1,i,128,1152]
        d['blk_b'].append(b)
    fw, fb = _fold_basic(dn['fuse'])
    d['dfuse'] = _lhsT_3x3(fw)                                # [4,5,128,1152]
    d['dfuse_b'] = fb
    # g2: 1x1, out channels c*9+t; regroup to per-tap tiles [t][c]
    g2w = _np(p['g2_w'])[:, :, 0, 0]                          # [1152, 512]
    g2w = g2w.reshape(128, 9, 512).transpose(1, 0, 2)         # [t, c, ci]
    arr = np.empty((4, 128, 9, 128), dtype=np.float32)        # [ci_t, 128ci, t, 128c]
    for t in range(9):
        m = g2w[t]                                            # [128c, 512ci]
        arr[:, :, t, :] = m.T.reshape(4, 128, 128)
    d['g2'] = np.ascontiguousarray(arr.reshape(4, 128, 9 * 128)).astype(bf16)
    d['g2_b'] = _np(p['g2_b']).reshape(128, 9)                # [c, t]
    d['bfuse'] = _lhsT_3x3(_np(p['fuse_w']))                  # [1,1,128,1152]
    d['bfuse_b'] = _np(p['fuse_b'])
    return d


# ---------------------------------------------------------------------------
# program builder
# ---------------------------------------------------------------------------
def _build_program():
    nc = bass.Bass()

    dram = {}
    def din(name, shape, dt=BF16):
        dram[name] = nc.dram_tensor(name, list(shape), dt, kind="ExternalInput")
        return dram[name]

    din("x", (4, 128, H, W))
    din("y", (4, 128, HP * WP))
    din("zeros", (128, XP * XP))
    din("wdown", (1, 4, 128, 128))
    for b in (1, 3, 5):
        din(f"g0_{b}", (4, 4, 128, 9 * 128))
        din(f"dsdown_{b}", (1, 4, 128, 128))
        for i in range(1, 5):
            din(f"blk{i}_{b}", (1, i, 128, 9 * 128))
        din(f"dfuse_{b}", (4, 5, 128, 9 * 128))
        din(f"g2_{b}", (4, 128, 9 * 128))
        din(f"bfuse_{b}", (1, 1, 128, 9 * 128))
    din("wfuse", (4, 4, 128, 9 * 128))

    # bias table [128, NB] fp32; column map built alongside the host table
    NB = 74
    din("bias", (128, NB), FP32)
    out_d = nc.dram_tensor("out", [4, 128, H * W], FP32, kind="ExternalOutput")

    bias_col = {}
    _next = [0]
    def bcol(key, n=1):
        if key not in bias_col:
            bias_col[key] = _next[0]
            _next[0] += n
        return bias_col[key]

    with tile.TileContext(nc) as tc, ExitStack() as ctx:
        persist = ctx.enter_context(tc.tile_pool(name="persist", bufs=1))
        bigpool = ctx.enter_context(tc.tile_pool(name="bigpool", bufs=4))
        ps = ctx.enter_context(tc.tile_pool(name="ps", bufs=6, space="PSUM"))

        xd74 = persist.tile([128, XP, XP], BF16, tag="xd74", name="xd74")
        r66 = [persist.tile([128, HP, WP], BF16, tag=f"r66_{i}", name=f"r66_{i}") for i in range(3)]
        dyn66 = persist.tile([128, HP, WP], BF16, tag="dyn66", name="dyn66")
        biasT = persist.tile([128, NB], FP32, tag="biasT", name="biasT")

        nc.sync.dma_start(biasT[:], dram["bias"][:])
        # zero pads of on-chip-written padded buffers
        nc.sync.dma_start(xd74[:].rearrange("p a b -> p (a b)"), dram["zeros"][:])
        for r in r66:
            nc.sync.dma_start(r[:].rearrange("p a b -> p (a b)"),
                              dram["zeros"][:, :HP * WP])
        nc.sync.dma_start(dyn66[:].rearrange("p a b -> p (a b)"),
                          dram["zeros"][:, :HP * WP])

        TAPS9 = [(t, t // 3 - 1, t % 3 - 1) for t in range(9)]
        TAP1 = [(0, 0, 0)]

        def conv_mms(psum, wc, taps, ci_list, dil=1):
            """Accumulate matmuls for one output tile.
            wc: weight chunk [128, CI, T, 128]; ci_list: (tile, pad, r0);
            taps: (weight_slot, dh_unit, dw_unit)."""
            n = len(ci_list) * len(taps)
            k = 0
            for ci_i, (st, pad, r0) in enumerate(ci_list):
                for t, dhu, dwu in taps:
                    dh, dw = dhu * dil, dwu * dil
                    rhs = st[:, r0 + pad + dh:r0 + pad + 8 + dh,
                             pad + dw:pad + dw + W]
                    k += 1
                    nc.tensor.matmul(psum[:], wc[:, ci_i, t, :], rhs,
                                     start=(k == 1), stop=(k == n))

        # ---------------- phase 0: top 1x1 down conv -> xd74 ----------------
        with tc.tile_pool(name="xpool", bufs=1) as xpool:
            xt = [xpool.tile([128, H, W], BF16, tag=f"x{i}", name=f"x{i}") for i in range(4)]
            for i in range(4):
                nc.sync.dma_start(xt[i][:].rearrange("p a b -> p (a b)"),
                                  dram["x"][i].rearrange("p a b -> p (a b)"))
            wd = xpool.tile([128, 4, 1, 128], BF16, tag="wd", name="wd")
            for ci in range(4):
                nc.sync.dma_start(wd[:, ci, 0, :], dram["wdown"][0, ci])
            cb = bcol("down")
            for s in range(8):
                r0 = s * 8
                psum = ps.tile([128, 8, W], FP32, tag="ps", name="psum")
                conv_mms(psum, wd, TAP1,
                         [(xt[ci], 0, r0) for ci in range(4)])
                nc.scalar.activation(xd74[:, r0 + 5:r0 + 13, 5:5 + W], psum[:],
                                     ACT_F.Identity, bias=biasT[:, cb:cb + 1])

        wpool = ctx.enter_context(tc.tile_pool(name="wpool", bufs=2))
        branch = ctx.enter_context(tc.tile_pool(name="branch", bufs=1))
        kp = ctx.enter_context(tc.tile_pool(name="kp", bufs=1))
        accp = ctx.enter_context(tc.tile_pool(name="accp", bufs=2))
        prodp = ctx.enter_context(tc.tile_pool(name="prodp", bufs=2))
        outp = ctx.enter_context(tc.tile_pool(name="outp", bufs=2))

        g0out = [branch.tile([128, HP, WP], BF16, tag=f"g0o{i}", name=f"g0o{i}") for i in range(4)]
        d66 = [branch.tile([128, HP, WP], BF16, tag=f"d66_{i}", name=f"d66_{i}") for i in range(5)]
        for tt in g0out + d66:
            nc.sync.dma_start(tt[:].rearrange("p a b -> p (a b)"),
                              dram["zeros"][:, :HP * WP])

        def load_chunk(name, co, CI, T):
            wc = wpool.tile([128, CI, T, 128], BF16, tag="W", name="wc")
            for ci in range(CI):
                nc.sync.dma_start(
                    wc[:, ci].rearrange("p a b -> p (a b)"),
                    dram[name][co, ci])
            return wc

        for bi, b in enumerate((1, 3, 5)):
            # ---- g0: 3x3 conv 512->512 on y ----
            yt = [bigpool.tile([128, HP, WP], BF16, tag="big", name="ybig") for _ in range(4)]
            for i in range(4):
                nc.sync.dma_start(yt[i][:].rearrange("p a b -> p (a b)"),
                                  dram["y"][i])
            bcol(f"g0_{b}", 4)
            for co in range(4):
                wc = load_chunk(f"g0_{b}", co, 4, 9)
                cb = bcol(f"g0_{b}") + co
                for s in range(8):
                    r0 = s * 8
                    psum = ps.tile([128, 8, W], FP32, tag="ps", name="psum")
                    conv_mms(psum, wc, TAPS9,
                             [(yt[ci], 1, r0) for ci in range(4)])
                    nc.scalar.activation(g0out[co][:, r0 + 1:r0 + 9, 1:1 + W],
                                         psum[:], ACT_F.Identity,
                                         bias=biasT[:, cb:cb + 1])

            # ---- dense down 1x1 512->128 ----
            wc = load_chunk(f"dsdown_{b}", 0, 4, 1)
            cb = bcol(f"dsdown_{b}")
            for s in range(8):
                r0 = s * 8
                psum = ps.tile([128, 8, W], FP32, tag="ps", name="psum")
                conv_mms(psum, wc, TAP1,
                         [(g0out[ci], 1, r0) for ci in range(4)])
                nc.scalar.activation(d66[0][:, r0 + 1:r0 + 9, 1:1 + W], psum[:],
                                     ACT_F.Identity, bias=biasT[:, cb:cb + 1])

            # ---- dense blocks ----
            for i in range(1, 5):
                wc = load_chunk(f"blk{i}_{b}", 0, i, 9)
                cb = bcol(f"blk{i}_{b}")
                srcs = [d66[j + 1] for j in range(i - 1)] + [d66[0]]
                for s in range(8):
                    r0 = s * 8
                    psum = ps.tile([128, 8, W], FP32, tag="ps", name="psum")
                    conv_mms(psum, wc, TAPS9,
                             [(t_, 1, r0) for t_ in srcs])
                    nc.scalar.activation(d66[i][:, r0 + 1:r0 + 9, 1:1 + W],
                                         psum[:], ACT_F.Relu,
                                         bias=biasT[:, cb:cb + 1])

            # ---- dense fuse 3x3 640->512 (materialize unpadded dfout) ----
            dfout = [bigpool.tile([128, H, W], BF16, tag="big", name="dfbig") for _ in range(4)]
            bcol(f"dfuse_{b}", 4)
            for co in range(4):
                wc = load_chunk(f"dfuse_{b}", co, 5, 9)
                cb = bcol(f"dfuse_{b}") + co
                for s in range(8):
                    r0 = s * 8
                    psum = ps.tile([128, 8, W], FP32, tag="ps", name="psum")
                    conv_mms(psum, wc, TAPS9,
                             [(g0out[ci], 1, r0) for ci in range(4)] +
                             [(d66[4], 1, r0)])
                    nc.scalar.activation(dfout[co][:, r0:r0 + 8, :], psum[:],
                                         ACT_F.Relu, bias=biasT[:, cb:cb + 1])

            # ---- g2 (1x1 -> 9 taps of 128ch) + dynamic conv, per stile ----
            wc_g2 = wpool.tile([128, 4, 9, 128], BF16, tag="W", name="wg2")
            for ci in range(4):
                nc.sync.dma_start(wc_g2[:, ci].rearrange("p a b -> p (a b)"),
                                  dram[f"g2_{b}"][ci])
            cbg2 = bcol(f"g2_{b}", 9)
            for s in range(8):
                r0 = s * 8
                kt = kp.tile([128, 9, 8, W], BF16, tag="k", name="kt")
                for t in range(9):
                    psum = ps.tile([128, 8, W], FP32, tag="ps", name="psum")
                    for ci in range(4):
                        nc.tensor.matmul(psum[:], wc_g2[:, ci, t, :],
                                         dfout[ci][:, r0:r0 + 8, :],
                                         start=(ci == 0), stop=(ci == 3))
                    nc.scalar.activation(kt[:, t], psum[:], ACT_F.Identity,
                                         bias=biasT[:, cbg2 + t:cbg2 + t + 1])
                acc = accp.tile([128, 8, W], FP32, tag="acc", name="acc")
                for t in range(9):
                    ki, kj = t // 3, t % 3
                    xs = xd74[:, r0 + 5 + (ki - 1) * b:r0 + 13 + (ki - 1) * b,
                              5 + (kj - 1) * b:5 + (kj - 1) * b + W]
                    if t == 0:
                        nc.vector.tensor_tensor(acc[:], xs, kt[:, t],
                                                op=AluOpType.mult)
                    else:
                        prod = prodp.tile([128, 8, W], FP32, tag="prod", name="prod")
                        nc.vector.tensor_tensor(prod[:], xs, kt[:, t],
                                                op=AluOpType.mult)
                        nc.vector.tensor_tensor(acc[:], acc[:], prod[:],
                                                op=AluOpType.add)
                nc.scalar.activation(dyn66[:, r0 + 1:r0 + 9, 1:1 + W], acc[:],
                                     ACT_F.Copy)

            # ---- branch fuse 3x3 128->128 -> r66[bi] ----
            wc = load_chunk(f"bfuse_{b}", 0, 1, 9)
            cb = bcol(f"bfuse_{b}")
            for s in range(8):
                r0 = s * 8
                psum = ps.tile([128, 8, W], FP32, tag="ps", name="psum")
                conv_mms(psum, wc, TAPS9, [(dyn66, 1, r0)])
                nc.scalar.activation(r66[bi][:, r0 + 1:r0 + 9, 1:1 + W], psum[:],
                                     ACT_F.Identity, bias=biasT[:, cb:cb + 1])

        # ---- final fuse 3x3 512->512 + BN + ReLU -> out ----
        bcol("wfuse", 4)
        for co in range(4):
            wc = load_chunk("wfuse", co, 4, 9)
            cb = bcol("wfuse") + co
            for s in range(8):
                r0 = s * 8
                psum = ps.tile([128, 8, W], FP32, tag="ps", name="psum")
                srcs = [(xd74, 5, r0), (r66[0], 1, r0), (r66[1], 1, r0),
                        (r66[2], 1, r0)]
                conv_mms(psum, wc, TAPS9, srcs)
                ot = outp.tile([128, 8, W], FP32, tag="out", name="ot")
                nc.scalar.activation(ot[:], psum[:], ACT_F.Relu,
                                     bias=biasT[:, cb:cb + 1])
                nc.sync.dma_start(out_d[co][:, r0 * W:(r0 + 8) * W],
                                  ot[:].rearrange("p a b -> p (a b)"))

    assert _next[0] <= NB, (_next[0], NB)
    _fix_sync_overflow(nc)
    return nc, bias_col, NB


_CACHE = {}


def _prepare(x, y, params):
    """Host-side prep: returns (nc, in_maps)."""
    x = np.asarray(x, dtype=np.float32)
    y = np.asarray(y, dtype=np.float32)
    N = x.shape[0]
    assert N == N_CORES

    if "nc" not in _CACHE:
        _CACHE["nc"], _CACHE["bias_col"], _CACHE["NB"] = _build_program()
    nc, bias_col, NB = _CACHE["nc"], _CACHE["bias_col"], _CACHE["NB"]

    # ---- host weight prep ----
    wd = _lhsT_1x1(_np(params['down_w']))                     # [1,4,128,128]
    down_b = _np(params['down_b'])
    branches = {b: _prep_branch(params[f'b{b}']) for b in (1, 3, 5)}
    fw, fb = _fold_basic(params['fuse'])
    wfuse = _lhsT_3x3(fw)

    bias_tab = np.zeros((128, NB), dtype=np.float32)
    def setb(key, vec):
        c = bias_col[key]
        if vec.ndim == 1:
            for j in range(vec.size // 128):
                bias_tab[:, c + j] = vec[j * 128:(j + 1) * 128]
        else:
            bias_tab[:, c:c + vec.shape[1]] = vec

    setb("down", down_b)
    for b in (1, 3, 5):
        d = branches[b]
        setb(f"g0_{b}", d['g0_b'])
        setb(f"dsdown_{b}", d['dsdown_b'])
        for i in range(1, 5):
            setb(f"blk{i}_{b}", d['blk_b'][i - 1])
        setb(f"dfuse_{b}", d['dfuse_b'])
        setb(f"g2_{b}", d['g2_b'])                            # [128, 9]
        setb(f"bfuse_{b}", d['bfuse_b'])
    setb("wfuse", fb)

    shared = {
        "zeros": np.zeros((128, XP * XP), dtype=bf16),
        "wdown": wd,
        "wfuse": wfuse,
        "bias": bias_tab,
    }
    for b in (1, 3, 5):
        d = branches[b]
        shared[f"g0_{b}"] = d['g0']
        shared[f"dsdown_{b}"] = d['dsdown']
        for i in range(1, 5):
            shared[f"blk{i}_{b}"] = d['blk'][i - 1]
        shared[f"dfuse_{b}"] = d['dfuse']
        shared[f"g2_{b}"] = d['g2']
        shared[f"bfuse_{b}"] = d['bfuse']

    yp = np.zeros((N, 512, HP, WP), dtype=np.float32)
    yp[:, :, 1:65, 1:65] = y
    yp = yp.astype(bf16).reshape(N, 4, 128, HP * WP)
    xb = x.astype(bf16).reshape(N, 4, 128, H, W)

    in_maps = []
    for n in range(N):
        m = dict(shared)
        m["x"] = np.ascontiguousarray(xb[n])
        m["y"] = np.ascontiguousarray(yp[n])
        in_maps.append(m)
    return nc, in_maps


def kernel(x, y, params):
    N = np.asarray(x).shape[0]
    nc, in_maps = _prepare(x, y, params)
    res = run_bass_kernel_spmd(nc, in_maps, core_ids=list(range(N_CORES)))
    _CACHE["last_res"] = res
    out = np.empty((N, 512, H, W), dtype=np.float32)
    for n in range(N):
        out[n] = res.results[n]["out"].reshape(512, H, W)
    return out


# revision 6
# speedup vs baseline: 1.0067x; 1.0067x over previous
"""Trainium2 Bass kernel for nn_DDPM_39831526703349.

Strategy: pure data parallel over batch N=8 -> 1 image per NeuronCore.
Each conv is lowered to per-tap matmuls over channel tiles with accumulation
in PSUM; BN is folded into weights host-side; bias/BN-shift/ReLU applied by
ScalarE on PSUM->SBUF eviction. Activations live in SBUF in a zero-padded
layout so 3x3 (and dilated) taps are pure AP shifts. The per-pixel dynamic
conv runs on VectorE against the padded xd buffer. Matmul operands are bf16
(fp32 accumulation in PSUM); biases and the final output are fp32.
"""

import os
import numpy as np
import ml_dtypes
from contextlib import ExitStack

import concourse.bass as bass
import concourse.mybir as mybir
import concourse.tile as tile
from concourse.alu_op_type import AluOpType
from concourse.bass_utils import run_bass_kernel_spmd

bf16 = ml_dtypes.bfloat16
FP32 = mybir.dt.float32
BF16 = mybir.dt.bfloat16
ACT_F = mybir.ActivationFunctionType

H = W = 64
HP = WP = 66          # pad-1 layout
XP = 74               # pad-5 layout for xd
EPS = 1e-5
N_CORES = 8


# ---------------------------------------------------------------------------
# sync-overflow post-pass: this walrus accepts at most one sync wait and one
# sync update per instruction; split extras into same-engine EventSemaphores.
# ---------------------------------------------------------------------------
def _fix_sync_overflow(nc, max_waits=1, max_updates=1):
    uid = 0
    n_split = 0
    for f in nc.m.functions:
        for bb in f.blocks:
            insts = list(bb.instructions)
            out = []
            changed = False
            for ins in insts:
                si = ins.sync_info
                waits = list(si.on_wait) if (si and si.on_wait) else []
                updates = list(si.on_update) if (si and si.on_update) else []
                if len(waits) <= max_waits and len(updates) <= max_updates:
                    out.append(ins)
                    continue
                changed = True
                n_split += 1
                keep_w = waits[-max_waits:] if max_waits else []
                excess_w = waits[:-max_waits] if max_waits else waits
                keep_u = updates[:max_updates]
                excess_u = updates[max_updates:]
                for wt in excess_w:
                    uid += 1
                    ev = mybir.InstEventSemaphore(name=f"syncfix_w_{ins.name}_{uid}")
                    ev.engine = ins.engine
                    ev.sync_info = mybir.SyncInfo(on_wait=[wt], on_update=[])
                    out.append(ev)
                ins.sync_info = mybir.SyncInfo(on_wait=keep_w, on_update=keep_u)
                out.append(ins)
                for ud in excess_u:
                    uid += 1
                    ev = mybir.InstEventSemaphore(name=f"syncfix_u_{ins.name}_{uid}")
                    ev.engine = ins.engine
                    ev.sync_info = mybir.SyncInfo(on_wait=[], on_update=[ud])
                    out.append(ev)
            if changed:
                bb.instructions = out
    return n_split


def _dedupe_ldweights(nc):
    """Delete an InstLdweights when the previous PE instruction stream since
    the last LDW consists only of matmuls and that LDW loaded the identical
    weights AP. LDWs carry no sem updates, so deletion only requires moving
    any waits onto the next PE instruction (syncfix splits overflow later)."""
    n_del = 0
    for f in nc.m.functions:
        for bb in f.blocks:
            insts = list(bb.instructions)
            out = []
            last_key = None
            only_mms = True
            pending_waits = []
            for ins in insts:
                eng = getattr(ins, 'engine', None)
                if isinstance(ins, mybir.InstLdweights):
                    ap = ins.ins[0]
                    key = repr(ap)
                    key = (key, getattr(ins, 'perf_mode', None),
                           getattr(ins, 'is_transpose', None))
                    if key == last_key and only_mms:
                        si = ins.sync_info
                        if si and si.on_wait:
                            pending_waits.extend(list(si.on_wait))
                        if si and si.on_update:
                            # unexpected: keep inst to preserve updates
                            out.append(ins)
                            continue
                        n_del += 1
                        continue
                    last_key = key
                    only_mms = True
                elif eng == mybir.EngineType.PE:
                    if not isinstance(ins, mybir.InstMatmult):
                        last_key = None
                        only_mms = True
                if pending_waits and eng == mybir.EngineType.PE:
                    si = ins.sync_info
                    w = list(si.on_wait) if (si and si.on_wait) else []
                    u = list(si.on_update) if (si and si.on_update) else []
                    ins.sync_info = mybir.SyncInfo(on_wait=pending_waits + w,
                                                   on_update=u)
                    pending_waits = []
                out.append(ins)
            assert not pending_waits
            bb.instructions = out
    return n_del


# ---------------------------------------------------------------------------
# host-side weight preparation
# ---------------------------------------------------------------------------
def _np(a):
    return np.asarray(a, dtype=np.float32)


def _fold_basic(p):
    """BasicConv2d: fold BN scale into conv weights; return (w_folded, bias)."""
    w = _np(p['w'])
    bnp = p['bn']
    inv = (1.0 / np.sqrt(_np(bnp['var']) + EPS)) * _np(bnp['gamma'])
    beta = _np(bnp['beta']) - _np(bnp['mean']) * inv
    return w * inv[:, None, None, None], beta


def _lhsT_3x3(w):
    """w [O, I, 3, 3] -> [co_t, ci_t, 128, 9*128] bf16 (lhsT per tap)."""
    O, I = w.shape[0], w.shape[1]
    full = w.transpose(1, 2, 3, 0).reshape(I, 9, O)          # [ci, t, co]
    arr = full.reshape(I // 128, 128, 9, O // 128, 128)
    arr = arr.transpose(3, 0, 1, 2, 4)                        # [co_t, ci_t, 128, 9, 128]
    return np.ascontiguousarray(arr.reshape(O // 128, I // 128, 128, 9 * 128)).astype(bf16)


def _lhsT_1x1(w):
    """w [O, I, 1, 1] -> [co_t, ci_t, 128, 128] bf16."""
    O, I = w.shape[0], w.shape[1]
    full = w[:, :, 0, 0].T                                    # [ci, co]
    arr = full.reshape(I // 128, 128, O // 128, 128).transpose(2, 0, 1, 3)
    return np.ascontiguousarray(arr).astype(bf16)


def _prep_branch(p):
    """Prepare one DepthDC branch's weights/biases."""
    d = {}
    g0w = _np(p['g0_w'])
    d['g0'] = _lhsT_3x3(g0w)                                  # [4,4,128,1152]
    d['g0_b'] = _np(p['g0_b'])
    dn = p['dense']
    d['dsdown'] = _lhsT_1x1(_np(dn['down_w']))                # [1,4,128,128]
    d['dsdown_b'] = _np(dn['down_b'])
    d['blk'] = []
    d['blk_b'] = []
    for bp in dn['blocks']:
        w, b = _fold_basic(bp)
        d['blk'].append(_lhsT_3x3(w))                         # [1,i,128,1152]
        d['blk_b'].append(b)
    fw, fb = _fold_basic(dn['fuse'])
    d['dfuse'] = _lhsT_3x3(fw)                                # [4,5,128,1152]
    d['dfuse_b'] = fb
    # g2: 1x1, out channels c*9+t; regroup to per-tap tiles [t][c]
    g2w = _np(p['g2_w'])[:, :, 0, 0]                          # [1152, 512]
    g2w = g2w.reshape(128, 9, 512).transpose(1, 0, 2)         # [t, c, ci]
    arr = np.empty((4, 128, 9, 128), dtype=np.float32)        # [ci_t, 128ci, t, 128c]
    for t in range(9):
        m = g2w[t]                                            # [128c, 512ci]
        arr[:, :, t, :] = m.T.reshape(4, 128, 128)
    d['g2'] = np.ascontiguousarray(arr.reshape(4, 128, 9 * 128)).astype(bf16)
    d['g2_b'] = _np(p['g2_b']).reshape(128, 9)                # [c, t]
    d['bfuse'] = _lhsT_3x3(_np(p['fuse_w']))                  # [1,1,128,1152]
    d['bfuse_b'] = _np(p['fuse_b'])
    return d


# ---------------------------------------------------------------------------
# program builder
# ---------------------------------------------------------------------------
def _build_program():
    nc = bass.Bass()

    dram = {}
    def din(name, shape, dt=BF16):
        dram[name] = nc.dram_tensor(name, list(shape), dt, kind="ExternalInput")
        return dram[name]

    din("x", (4, 128, H, W))
    din("y", (4, 128, HP * WP))
    din("zeros", (128, XP * XP))
    din("wdown", (1, 4, 128, 128))
    for b in (1, 3, 5):
        din(f"g0_{b}", (4, 4, 128, 9 * 128))
        din(f"dsdown_{b}", (1, 4, 128, 128))
        for i in range(1, 5):
            din(f"blk{i}_{b}", (1, i, 128, 9 * 128))
        din(f"dfuse_{b}", (4, 5, 128, 9 * 128))
        din(f"g2_{b}", (4, 128, 9 * 128))
        din(f"bfuse_{b}", (1, 1, 128, 9 * 128))
    din("wfuse", (4, 4, 128, 9 * 128))

    # bias table [128, NB] fp32; column map built alongside the host table
    NB = 74
    din("bias", (128, NB), FP32)
    out_d = nc.dram_tensor("out", [4, 128, H * W], FP32, kind="ExternalOutput")

    bias_col = {}
    _next = [0]
    def bcol(key, n=1):
        if key not in bias_col:
            bias_col[key] = _next[0]
            _next[0] += n
        return bias_col[key]

    with tile.TileContext(nc) as tc, ExitStack() as ctx:
        persist = ctx.enter_context(tc.tile_pool(name="persist", bufs=1))
        bigpool = ctx.enter_context(tc.tile_pool(name="bigpool", bufs=4))
        ps = ctx.enter_context(tc.tile_pool(name="ps", bufs=6, space="PSUM"))

        xd74 = persist.tile([128, XP, XP], BF16, tag="xd74", name="xd74")
        r66 = [persist.tile([128, HP, WP], BF16, tag=f"r66_{i}", name=f"r66_{i}") for i in range(3)]
        dyn66 = persist.tile([128, HP, WP], BF16, tag="dyn66", name="dyn66")
        biasT = persist.tile([128, NB], FP32, tag="biasT", name="biasT")

        nc.sync.dma_start(biasT[:], dram["bias"][:])
        # zero pads of on-chip-written padded buffers
        nc.sync.dma_start(xd74[:].rearrange("p a b -> p (a b)"), dram["zeros"][:])
        for r in r66:
            nc.sync.dma_start(r[:].rearrange("p a b -> p (a b)"),
                              dram["zeros"][:, :HP * WP])
        nc.sync.dma_start(dyn66[:].rearrange("p a b -> p (a b)"),
                          dram["zeros"][:, :HP * WP])

        TAPS9 = [(t, t // 3 - 1, t % 3 - 1) for t in range(9)]
        TAP1 = [(0, 0, 0)]

        def conv_mms(psum, wc, taps, ci_list, dil=1):
            """Accumulate matmuls for one output tile.
            wc: weight chunk [128, CI, T, 128]; ci_list: (tile, pad, r0);
            taps: (weight_slot, dh_unit, dw_unit)."""
            n = len(ci_list) * len(taps)
            k = 0
            for ci_i, (st, pad, r0) in enumerate(ci_list):
                for t, dhu, dwu in taps:
                    dh, dw = dhu * dil, dwu * dil
                    rhs = st[:, r0 + pad + dh:r0 + pad + 8 + dh,
                             pad + dw:pad + dw + W]
                    k += 1
                    nc.tensor.matmul(psum[:], wc[:, ci_i, t, :], rhs,
                                     start=(k == 1), stop=(k == n))

        def conv_grouped(wc, taps, srcs, evict, dil=1, group=4):
            """Weight-stationary conv over all 8 stiles in groups.
            srcs: list of (tile, pad); evict(s, psum) writes output for
            stile s. Each weight (ci, tap) is loaded once per group and
            used for `group` consecutive matmuls -> redundant LDWs get
            deduped post-hoc."""
            n = len(srcs) * len(taps)
            for sg in range(8 // group):
                psums = [ps.tile([128, 8, W], FP32, tag="ps", name="psum")
                         for _ in range(group)]
                k = 0
                for ci_i, (st, pad) in enumerate(srcs):
                    for t, dhu, dwu in taps:
                        dh, dw = dhu * dil, dwu * dil
                        k += 1
                        for s4 in range(group):
                            r0 = (sg * group + s4) * 8
                            rhs = st[:, r0 + pad + dh:r0 + pad + 8 + dh,
                                     pad + dw:pad + dw + W]
                            nc.tensor.matmul(psums[s4][:], wc[:, ci_i, t, :],
                                             rhs, start=(k == 1), stop=(k == n))
                for s4 in range(group):
                    evict(sg * group + s4, psums[s4])

        # ---------------- phase 0: top 1x1 down conv -> xd74 ----------------
        with tc.tile_pool(name="xpool", bufs=1) as xpool:
            xt = [xpool.tile([128, H, W], BF16, tag=f"x{i}", name=f"x{i}") for i in range(4)]
            for i in range(4):
                nc.sync.dma_start(xt[i][:].rearrange("p a b -> p (a b)"),
                                  dram["x"][i].rearrange("p a b -> p (a b)"))
            wd = xpool.tile([128, 4, 1, 128], BF16, tag="wd", name="wd")
            for ci in range(4):
                nc.sync.dma_start(wd[:, ci, 0, :], dram["wdown"][0, ci])
            cb = bcol("down")
            for s in range(8):
                r0 = s * 8
                psum = ps.tile([128, 8, W], FP32, tag="ps", name="psum")
                conv_mms(psum, wd, TAP1,
                         [(xt[ci], 0, r0) for ci in range(4)])
                nc.scalar.activation(xd74[:, r0 + 5:r0 + 13, 5:5 + W], psum[:],
                                     ACT_F.Identity, bias=biasT[:, cb:cb + 1])

        wpool = ctx.enter_context(tc.tile_pool(name="wpool", bufs=2))
        branch = ctx.enter_context(tc.tile_pool(name="branch", bufs=1))
        kp = ctx.enter_context(tc.tile_pool(name="kp", bufs=1))
        accp = ctx.enter_context(tc.tile_pool(name="accp", bufs=2))
        prodp = ctx.enter_context(tc.tile_pool(name="prodp", bufs=2))
        outp = ctx.enter_context(tc.tile_pool(name="outp", bufs=2))

        g0out = [branch.tile([128, HP, WP], BF16, tag=f"g0o{i}", name=f"g0o{i}") for i in range(4)]
        d66 = [branch.tile([128, HP, WP], BF16, tag=f"d66_{i}", name=f"d66_{i}") for i in range(5)]
        for tt in g0out + d66:
            nc.sync.dma_start(tt[:].rearrange("p a b -> p (a b)"),
                              dram["zeros"][:, :HP * WP])

        def load_chunk(name, co, CI, T):
            wc = wpool.tile([128, CI, T, 128], BF16, tag="W", name="wc")
            for ci in range(CI):
                nc.sync.dma_start(
                    wc[:, ci].rearrange("p a b -> p (a b)"),
                    dram[name][co, ci])
            return wc

        for bi, b in enumerate((1, 3, 5)):
            # ---- g0: 3x3 conv 512->512 on y ----
            yt = [bigpool.tile([128, HP, WP], BF16, tag="big", name="ybig") for _ in range(4)]
            for i in range(4):
                nc.sync.dma_start(yt[i][:].rearrange("p a b -> p (a b)"),
                                  dram["y"][i])
            bcol(f"g0_{b}", 4)
            for co in range(4):
                wc = load_chunk(f"g0_{b}", co, 4, 9)
                cb = bcol(f"g0_{b}") + co
                def ev_g0(s, psum, co=co, cb=cb):
                    r0 = s * 8
                    nc.scalar.activation(g0out[co][:, r0 + 1:r0 + 9, 1:1 + W],
                                         psum[:], ACT_F.Identity,
                                         bias=biasT[:, cb:cb + 1])
                conv_grouped(wc, TAPS9, [(yt[ci], 1) for ci in range(4)], ev_g0)

            # ---- dense down 1x1 512->128 ----
            wc = load_chunk(f"dsdown_{b}", 0, 4, 1)
            cb = bcol(f"dsdown_{b}")
            for s in range(8):
                r0 = s * 8
                psum = ps.tile([128, 8, W], FP32, tag="ps", name="psum")
                conv_mms(psum, wc, TAP1,
                         [(g0out[ci], 1, r0) for ci in range(4)])
                nc.scalar.activation(d66[0][:, r0 + 1:r0 + 9, 1:1 + W], psum[:],
                                     ACT_F.Identity, bias=biasT[:, cb:cb + 1])

            # ---- dense blocks ----
            for i in range(1, 5):
                wc = load_chunk(f"blk{i}_{b}", 0, i, 9)
                cb = bcol(f"blk{i}_{b}")
                srcs = [d66[j + 1] for j in range(i - 1)] + [d66[0]]
                def ev_blk(s, psum, i=i, cb=cb):
                    r0 = s * 8
                    nc.scalar.activation(d66[i][:, r0 + 1:r0 + 9, 1:1 + W],
                                         psum[:], ACT_F.Relu,
                                         bias=biasT[:, cb:cb + 1])
                conv_grouped(wc, TAPS9, [(t_, 1) for t_ in srcs], ev_blk)

            # ---- dense fuse 3x3 640->512 (materialize unpadded dfout) ----
            dfout = [bigpool.tile([128, H, W], BF16, tag="big", name="dfbig") for _ in range(4)]
            bcol(f"dfuse_{b}", 4)
            for co in range(4):
                wc = load_chunk(f"dfuse_{b}", co, 5, 9)
                cb = bcol(f"dfuse_{b}") + co
                def ev_df(s, psum, co=co, cb=cb):
                    r0 = s * 8
                    nc.scalar.activation(dfout[co][:, r0:r0 + 8, :], psum[:],
                                         ACT_F.Relu, bias=biasT[:, cb:cb + 1])
                conv_grouped(wc, TAPS9,
                             [(g0out[ci], 1) for ci in range(4)] +
                             [(d66[4], 1)], ev_df)

            # ---- g2 (1x1 -> 9 taps of 128ch) + dynamic conv, per stile ----
            wc_g2 = wpool.tile([128, 4, 9, 128], BF16, tag="W", name="wg2")
            for ci in range(4):
                nc.sync.dma_start(wc_g2[:, ci].rearrange("p a b -> p (a b)"),
                                  dram[f"g2_{b}"][ci])
            cbg2 = bcol(f"g2_{b}", 9)
            for s in range(8):
                r0 = s * 8
                kt = kp.tile([128, 9, 8, W], BF16, tag="k", name="kt")
                for t in range(9):
                    psum = ps.tile([128, 8, W], FP32, tag="ps", name="psum")
                    for ci in range(4):
                        nc.tensor.matmul(psum[:], wc_g2[:, ci, t, :],
                                         dfout[ci][:, r0:r0 + 8, :],
                                         start=(ci == 0), stop=(ci == 3))
                    nc.scalar.activation(kt[:, t], psum[:], ACT_F.Identity,
                                         bias=biasT[:, cbg2 + t:cbg2 + t + 1])
                acc = accp.tile([128, 8, W], FP32, tag="acc", name="acc")
                for t in range(9):
                    ki, kj = t // 3, t % 3
                    xs = xd74[:, r0 + 5 + (ki - 1) * b:r0 + 13 + (ki - 1) * b,
                              5 + (kj - 1) * b:5 + (kj - 1) * b + W]
                    if t == 0:
                        nc.vector.tensor_tensor(acc[:], xs, kt[:, t],
                                                op=AluOpType.mult)
                    else:
                        prod = prodp.tile([128, 8, W], FP32, tag="prod", name="prod")
                        nc.vector.tensor_tensor(prod[:], xs, kt[:, t],
                                                op=AluOpType.mult)
                        nc.vector.tensor_tensor(acc[:], acc[:], prod[:],
                                                op=AluOpType.add)
                nc.scalar.activation(dyn66[:, r0 + 1:r0 + 9, 1:1 + W], acc[:],
                                     ACT_F.Copy)

            # ---- branch fuse 3x3 128->128 -> r66[bi] ----
            wc = load_chunk(f"bfuse_{b}", 0, 1, 9)
            cb = bcol(f"bfuse_{b}")
            def ev_bf(s, psum, bi=bi, cb=cb):
                r0 = s * 8
                nc.scalar.activation(r66[bi][:, r0 + 1:r0 + 9, 1:1 + W], psum[:],
                                     ACT_F.Identity, bias=biasT[:, cb:cb + 1])
            conv_grouped(wc, TAPS9, [(dyn66, 1)], ev_bf)

        # ---- final fuse 3x3 512->512 + BN + ReLU -> out ----
        bcol("wfuse", 4)
        for co in range(4):
            wc = load_chunk("wfuse", co, 4, 9)
            cb = bcol("wfuse") + co
            def ev_ff(s, psum, co=co, cb=cb):
                r0 = s * 8
                ot = outp.tile([128, 8, W], FP32, tag="out", name="ot")
                nc.scalar.activation(ot[:], psum[:], ACT_F.Relu,
                                     bias=biasT[:, cb:cb + 1])
                nc.sync.dma_start(out_d[co][:, r0 * W:(r0 + 8) * W],
                                  ot[:].rearrange("p a b -> p (a b)"))
            conv_grouped(wc, TAPS9,
                         [(xd74, 5), (r66[0], 1), (r66[1], 1), (r66[2], 1)],
                         ev_ff)

    assert _next[0] <= NB, (_next[0], NB)
    n_ldw = _dedupe_ldweights(nc)
    n_sync = _fix_sync_overflow(nc)
    if os.environ.get("KERNEL_VERBOSE"):
        print(f"deduped {n_ldw} ldweights, split {n_sync} sync overflows")
    return nc, bias_col, NB


_CACHE = {}


def _prepare(x, y, params):
    """Host-side prep: returns (nc, in_maps)."""
    x = np.asarray(x, dtype=np.float32)
    y = np.asarray(y, dtype=np.float32)
    N = x.shape[0]
    assert N == N_CORES

    if "nc" not in _CACHE:
        _CACHE["nc"], _CACHE["bias_col"], _CACHE["NB"] = _build_program()
    nc, bias_col, NB = _CACHE["nc"], _CACHE["bias_col"], _CACHE["NB"]

    # ---- host weight prep ----
    wd = _lhsT_1x1(_np(params['down_w']))                     # [1,4,128,128]
    down_b = _np(params['down_b'])
    branches = {b: _prep_branch(params[f'b{b}']) for b in (1, 3, 5)}
    fw, fb = _fold_basic(params['fuse'])
    wfuse = _lhsT_3x3(fw)

    bias_tab = np.zeros((128, NB), dtype=np.float32)
    def setb(key, vec):
        c = bias_col[key]
        if vec.ndim == 1:
            for j in range(vec.size // 128):
                bias_tab[:, c + j] = vec[j * 128:(j + 1) * 128]
        else:
            bias_tab[:, c:c + vec.shape[1]] = vec

    setb("down", down_b)
    for b in (1, 3, 5):
        d = branches[b]
        setb(f"g0_{b}", d['g0_b'])
        setb(f"dsdown_{b}", d['dsdown_b'])
        for i in range(1, 5):
            setb(f"blk{i}_{b}", d['blk_b'][i - 1])
        setb(f"dfuse_{b}", d['dfuse_b'])
        setb(f"g2_{b}", d['g2_b'])                            # [128, 9]
        setb(f"bfuse_{b}", d['bfuse_b'])
    setb("wfuse", fb)

    shared = {
        "zeros": np.zeros((128, XP * XP), dtype=bf16),
        "wdown": wd,
        "wfuse": wfuse,
        "bias": bias_tab,
    }
    for b in (1, 3, 5):
        d = branches[b]
        shared[f"g0_{b}"] = d['g0']
        shared[f"dsdown_{b}"] = d['dsdown']
        for i in range(1, 5):
            shared[f"blk{i}_{b}"] = d['blk'][i - 1]
        shared[f"dfuse_{b}"] = d['dfuse']
        shared[f"g2_{b}"] = d['g2']
        shared[f"bfuse_{b}"] = d['bfuse']

    yp = np.zeros((N, 512, HP, WP), dtype=np.float32)
    yp[:, :, 1:65, 1:65] = y
    yp = yp.astype(bf16).reshape(N, 4, 128, HP * WP)
    xb = x.astype(bf16).reshape(N, 4, 128, H, W)

    in_maps = []
    for n in range(N):
        m = dict(shared)
        m["x"] = np.ascontiguousarray(xb[n])
        m["y"] = np.ascontiguousarray(yp[n])
        in_maps.append(m)
    return nc, in_maps


def kernel(x, y, params):
    N = np.asarray(x).shape[0]
    nc, in_maps = _prepare(x, y, params)
    res = run_bass_kernel_spmd(nc, in_maps, core_ids=list(range(N_CORES)))
    _CACHE["last_res"] = res
    out = np.empty((N, 512, H, W), dtype=np.float32)
    for n in range(N):
        out[n] = res.results[n]["out"].reshape(512, H, W)
    return out
